# revision 9
# baseline (speedup 1.0000x reference)
"""Trainium2 Bass kernel for CIN: out[b,m,d] = sigmoid(einsum('bid,bjd,ijm', x0, x, K)).

v9: v8 + cross-group software pipelining of the GPSIMD-routed chunks.

The evac routes with long latency (ACT copy -> GPSIMD multiply, ~1.9 us)
stall the PE when their contraction matmuls sit in the same group's
in-order PE stream (v8 lost ~1.45 us/group to this). Here each group's
5 GP-routed contraction matmuls are EMITTED DURING THE NEXT GROUP's
stream, giving the route a full group-period of slack; the PE then never
waits on them. PSUM accumulators (pso, bufs=3) stay live across the
boundary; start fires on the group's first DVE-chunk matmul and stop on
its last GP-chunk matmul one group later.

Other structure as v8: all matmuls bf16 K=120 uniform (mixed row-configs
serialize the PE), 14 replication MMs broadcast x0 rows via 0/1
matrices, evac split DVE-direct(8) / ACT-copy+DVE-2x(1) /
ACT-copy+GPSIMD(5), ACT sigmoid evacuation.
"""

import sys

for _p in ("/opt/trn_rl_repo", "/root/.axon_site/_ro/trn_rl_repo"):
    if _p not in sys.path:
        sys.path.insert(0, _p)

from contextlib import ExitStack

import numpy as np
import ml_dtypes

import concourse.bass as bass
from concourse import bacc
import concourse.tile as tile
from concourse import mybir
from concourse.bass_utils import run_bass_kernel_spmd

B, F0, F, D, M = 4096, 40, 40, 64, 128
NCORES = 8
NB = B // NCORES            # 512
GB = 8
FREE = GB * D               # 512
NG = NB // GB               # 64
NCH = 14
CONTRACT = 120

f32 = mybir.dt.float32
bf16 = mybir.dt.bfloat16
BF16NP = ml_dtypes.bfloat16

GP_CHUNKS = [9, 10, 11, 12, 13]     # ACT-copy + GPSIMD; reals deferred 1 group
AD_CHUNK = 8                        # ACT-copy + DVE 2x; real last in-group
DV_REALS = [0, 1, 2, 3, 4, 5, 6, 7, AD_CHUNK]
LOOKAHEAD = 4


def _pack_consts(kernel_np: np.ndarray):
    kc = np.zeros((CONTRACT, NCH, M), np.float32)
    pp = np.zeros((CONTRACT, NCH, 128), np.float32)
    for c in range(NCH):
        for t in range(3):
            i = 3 * c + t
            if i < F0:
                kc[t * 40:(t + 1) * 40, c, :] = kernel_np[i]
                pp[i, c, t * 40:(t + 1) * 40] = 1.0
    return kc.astype(BF16NP), pp.astype(BF16NP)


def _build():
    nc = bacc.Bacc("TRN2", num_devices=NCORES)
    x0h = nc.declare_dram_parameter("x0h", [F0, NG, FREE], bf16, isOutput=False)
    xh = nc.declare_dram_parameter("xh", [F, NG, FREE], bf16, isOutput=False)
    kch = nc.declare_dram_parameter("kch", [CONTRACT, NCH, M], bf16, isOutput=False)
    pph = nc.declare_dram_parameter("pph", [CONTRACT, NCH, 128], bf16, isOutput=False)
    outp = nc.declare_dram_parameter("out", [NB, M, D], f32, isOutput=True)

    with ExitStack() as ctx:
        tc = ctx.enter_context(tile.TileContext(nc))
        singles = ctx.enter_context(tc.tile_pool(name="singles", bufs=1))
        xr_pool = ctx.enter_context(tc.tile_pool(name="xr", bufs=3))
        x0_pool = ctx.enter_context(tc.tile_pool(name="x0", bufs=3))
        sb_pool = ctx.enter_context(tc.tile_pool(name="sb", bufs=12))
        a_pool = ctx.enter_context(tc.tile_pool(name="a", bufs=28))
        osb_pool = ctx.enter_context(tc.tile_pool(name="osb", bufs=3))
        psa_pool = ctx.enter_context(tc.tile_pool(name="psa", bufs=5, space="PSUM"))
        pso_pool = ctx.enter_context(tc.tile_pool(name="pso", bufs=3, space="PSUM"))

        kcw = singles.tile([CONTRACT, NCH, M], bf16)
        nc.sync.dma_start(out=kcw, in_=kch[:, :, :])
        ppw = singles.tile([CONTRACT, NCH, 128], bf16)
        nc.sync.dma_start(out=ppw, in_=pph[:, :, :])

        prev = None     # (g, pso, acs) of the previous group

        def finish_group(state):
            g, pso, acs = state
            for j, c in enumerate(GP_CHUNKS):
                nc.tensor.matmul(pso, kcw[:, c, :], acs[c],
                                 start=False, stop=(j == len(GP_CHUNKS) - 1))
            bsl = slice(g * GB, (g + 1) * GB)
            osb = osb_pool.tile([M, GB, D], f32, tag="osb")
            nc.scalar.activation(osb.rearrange("m b d -> m (b d)"), pso,
                                 mybir.ActivationFunctionType.Sigmoid)
            nc.scalar.dma_start(out=outp[bsl].transpose([1, 0, 2]), in_=osb)

        for g in range(NG):
            xr = xr_pool.tile([CONTRACT, FREE], bf16, tag="xr")
            for r in range(3):
                nc.sync.dma_start(out=xr[r * 40:(r + 1) * 40, :], in_=xh[:, g, :])
            # x0t rows 0-39 real; rows 40-119 finite copies (zero-weighted)
            x0t = x0_pool.tile([CONTRACT, FREE], bf16, tag="x0t")
            for r in range(3):
                nc.sync.dma_start(out=x0t[r * 40:(r + 1) * 40, :], in_=x0h[:, g, :])

            pso = pso_pool.tile([M, FREE], f32, tag="pso")
            acs = [None] * NCH
            for k in range(NCH):
                c = k
                psa = psa_pool.tile([128, FREE], f32, tag="psa")
                nc.tensor.matmul(psa, ppw[:, c, :], x0t, start=True, stop=True)
                ac = a_pool.tile([CONTRACT, FREE], bf16, tag="ac")
                if c in GP_CHUNKS or c == AD_CHUNK:
                    sba = sb_pool.tile([CONTRACT, FREE], bf16, tag="sba")
                    nc.scalar.copy(sba, psa[0:CONTRACT, :])
                    eng = nc.gpsimd if c in GP_CHUNKS else nc.vector
                    eng.tensor_tensor(out=ac, in0=sba, in1=xr,
                                      op=mybir.AluOpType.mult)
                else:
                    nc.vector.tensor_tensor(out=ac, in0=psa[0:CONTRACT, :],
                                            in1=xr, op=mybir.AluOpType.mult)
                acs[c] = ac
                if k >= LOOKAHEAD and (k - LOOKAHEAD) < len(DV_REALS):
                    cc = DV_REALS[k - LOOKAHEAD]
                    nc.tensor.matmul(pso, kcw[:, cc, :], acs[cc],
                                     start=(cc == DV_REALS[0]), stop=False)
            for k in range(NCH, LOOKAHEAD + len(DV_REALS)):
                cc = DV_REALS[k - LOOKAHEAD]
                nc.tensor.matmul(pso, kcw[:, cc, :], acs[cc],
                                 start=(cc == DV_REALS[0]), stop=False)

            if prev is not None:
                finish_group(prev)
            prev = (g, pso, acs)

        finish_group(prev)

    nc.finalize()
    return nc


_NC_CACHE = {}


def _get_nc():
    if "nc" not in _NC_CACHE:
        _NC_CACHE["nc"] = _build()
    return _NC_CACHE["nc"]


def _in_maps(x0: np.ndarray, x: np.ndarray, kernel_np: np.ndarray):
    kc, pp = _pack_consts(np.asarray(kernel_np, dtype=np.float32))
    x0 = np.asarray(x0, dtype=np.float32).astype(BF16NP)
    x = np.asarray(x, dtype=np.float32).astype(BF16NP)
    maps = []
    for i in range(NCORES):
        sl = slice(i * NB, (i + 1) * NB)
        x0c = np.ascontiguousarray(
            x0[sl].transpose(1, 0, 2).reshape(F0, NG, FREE))
        xc = np.ascontiguousarray(
            x[sl].transpose(1, 0, 2).reshape(F, NG, FREE))
        maps.append({"x0h": x0c, "xh": xc, "kch": kc, "pph": pp})
    return maps


def kernel(x0: np.ndarray, x: np.ndarray, kernel: np.ndarray) -> np.ndarray:
    nc = _get_nc()
    in_maps = _in_maps(x0, x, kernel)
    res = run_bass_kernel_spmd(nc, in_maps, list(range(NCORES)))
    out = np.concatenate([np.asarray(r["out"]) for r in res.results], axis=0)
    return out.astype(np.float32)


# revision 11
# speedup vs baseline: 1.6375x; 1.6375x over previous
"""Trainium2 Bass kernel for CIN: out[b,m,d] = sigmoid(einsum('bid,bjd,ijm', x0, x, K)).

v10: uniform-K bf16 replication, engine-balanced evacuation.

Microbenchmark-derived rules baked in:
  - PE matmuls stream at ~216 ns/MM (N=512 warm) only when every MM has
    the SAME contraction row-config and dtype; K-mixing serializes to
    ~540 ns, fp32 moving operands pump at half rate AND lower to 2 HW
    passes. So: every matmul is bf16 with K=120.
  - PSUM-sourced elementwise ops run at 1x: DVE tensor_tensor 690 ns,
    ACT copy 720 ns per [*,512] chunk. The per-group broadcast evac
    (14 chunks) is split: 8 chunks DVE-direct, 5 chunks ACT-copy +
    GPSIMD-multiply, 1 chunk ACT-copy + DVE 2x multiply.

Per group (8 b's, free=512): 14 bf16 K=120 replication MMs (psa =
pp_c.T @ x0t broadcasts x0 rows across partitions), per-chunk multiply
ac = psa * xr on DVE/GP, 14 bf16 K=120 contraction MMs accumulate, ACT
sigmoid evacuation, DMA out. PE: 28 MMs/group = the pacer (~6 us).
"""

import sys

for _p in ("/opt/trn_rl_repo", "/root/.axon_site/_ro/trn_rl_repo"):
    if _p not in sys.path:
        sys.path.insert(0, _p)

from contextlib import ExitStack

import numpy as np
import ml_dtypes

import concourse.bass as bass
from concourse import bacc
import concourse.tile as tile
from concourse import mybir
from concourse.bass_utils import run_bass_kernel_spmd

B, F0, F, D, M = 4096, 40, 40, 64, 128
NCORES = 8
NB = B // NCORES            # 512
GB = 8
FREE = GB * D               # 512
NG = NB // GB               # 64
NCH = 14
CONTRACT = 120

f32 = mybir.dt.float32
bf16 = mybir.dt.bfloat16
BF16NP = ml_dtypes.bfloat16

GP_ROUTE = {2, 5, 8, 11}        # ACT-copy + GPSIMD multiply
ACTDVE_ROUTE = {4, 13}          # ACT-copy + DVE 2x multiply
LOOKAHEAD = 3


def _pack_consts(kernel_np: np.ndarray):
    kc = np.zeros((CONTRACT, NCH, M), np.float32)
    pp = np.zeros((CONTRACT, NCH, 128), np.float32)
    for c in range(NCH):
        for t in range(3):
            i = 3 * c + t
            if i < F0:
                kc[t * 40:(t + 1) * 40, c, :] = kernel_np[i]
                pp[i, c, t * 40:(t + 1) * 40] = 1.0
    return kc.astype(BF16NP), pp.astype(BF16NP)


def _build():
    nc = bacc.Bacc("TRN2", num_devices=NCORES)
    x0h = nc.declare_dram_parameter("x0h", [F0, NG, FREE], bf16, isOutput=False)
    xh = nc.declare_dram_parameter("xh", [F, NG, FREE], bf16, isOutput=False)
    kch = nc.declare_dram_parameter("kch", [CONTRACT, NCH, M], bf16, isOutput=False)
    pph = nc.declare_dram_parameter("pph", [CONTRACT, NCH, 128], bf16, isOutput=False)
    outp = nc.declare_dram_parameter("out", [NB, M, D], f32, isOutput=True)

    with ExitStack() as ctx:
        tc = ctx.enter_context(tile.TileContext(nc))
        singles = ctx.enter_context(tc.tile_pool(name="singles", bufs=1))
        xr_pool = ctx.enter_context(tc.tile_pool(name="xr", bufs=3))
        x0_pool = ctx.enter_context(tc.tile_pool(name="x0", bufs=3))
        sb_pool = ctx.enter_context(tc.tile_pool(name="sb", bufs=6))
        a_pool = ctx.enter_context(tc.tile_pool(name="a", bufs=6))
        osb_pool = ctx.enter_context(tc.tile_pool(name="osb", bufs=3))
        psa_pool = ctx.enter_context(tc.tile_pool(name="psa", bufs=5, space="PSUM"))
        pso_pool = ctx.enter_context(tc.tile_pool(name="pso", bufs=3, space="PSUM"))

        kcw = singles.tile([CONTRACT, NCH, M], bf16)
        nc.sync.dma_start(out=kcw, in_=kch[:, :, :])
        ppw = singles.tile([CONTRACT, NCH, 128], bf16)
        nc.sync.dma_start(out=ppw, in_=pph[:, :, :])

        for g in range(NG):
            bsl = slice(g * GB, (g + 1) * GB)
            xr = xr_pool.tile([CONTRACT, FREE], bf16, tag="xr")
            for r in range(3):
                nc.sync.dma_start(out=xr[r * 40:(r + 1) * 40, :], in_=xh[:, g, :])
            # x0t rows 0-39 real; rows 40-119 finite copies (zero-weighted)
            x0t = x0_pool.tile([CONTRACT, FREE], bf16, tag="x0t")
            for r in range(3):
                nc.sync.dma_start(out=x0t[r * 40:(r + 1) * 40, :], in_=x0h[:, g, :])

            pso = pso_pool.tile([M, FREE], f32, tag="pso")
            acs = [None] * NCH
            for c in range(NCH + LOOKAHEAD):
                if c < NCH:
                    psa = psa_pool.tile([128, FREE], f32, tag="psa")
                    nc.tensor.matmul(psa, ppw[:, c, :], x0t, start=True, stop=True)
                    ac = a_pool.tile([CONTRACT, FREE], bf16, tag="ac")
                    if c in GP_ROUTE or c in ACTDVE_ROUTE:
                        sba = sb_pool.tile([CONTRACT, FREE], bf16, tag="sba")
                        nc.scalar.copy(sba, psa[0:CONTRACT, :])
                        eng = nc.gpsimd if c in GP_ROUTE else nc.vector
                        eng.tensor_tensor(out=ac, in0=sba, in1=xr,
                                          op=mybir.AluOpType.mult)
                    else:
                        nc.vector.tensor_tensor(out=ac, in0=psa[0:CONTRACT, :],
                                                in1=xr, op=mybir.AluOpType.mult)
                    acs[c] = ac
                if c >= LOOKAHEAD:
                    cc = c - LOOKAHEAD
                    nc.tensor.matmul(pso, kcw[:, cc, :], acs[cc],
                                     start=(cc == 0), stop=(cc == NCH - 1))

            osb = osb_pool.tile([M, GB, D], f32, tag="osb")
            nc.scalar.activation(osb.rearrange("m b d -> m (b d)"), pso,
                                 mybir.ActivationFunctionType.Sigmoid)
            nc.sync.dma_start(out=outp[bsl].transpose([1, 0, 2]), in_=osb)

    nc.finalize()
    return nc


_NC_CACHE = {}


def _get_nc():
    if "nc" not in _NC_CACHE:
        _NC_CACHE["nc"] = _build()
    return _NC_CACHE["nc"]


def _in_maps(x0: np.ndarray, x: np.ndarray, kernel_np: np.ndarray):
    kc, pp = _pack_consts(np.asarray(kernel_np, dtype=np.float32))
    x0 = np.asarray(x0, dtype=np.float32).astype(BF16NP)
    x = np.asarray(x, dtype=np.float32).astype(BF16NP)
    maps = []
    for i in range(NCORES):
        sl = slice(i * NB, (i + 1) * NB)
        x0c = np.ascontiguousarray(
            x0[sl].transpose(1, 0, 2).reshape(F0, NG, FREE))
        xc = np.ascontiguousarray(
            x[sl].transpose(1, 0, 2).reshape(F, NG, FREE))
        maps.append({"x0h": x0c, "xh": xc, "kch": kc, "pph": pp})
    return maps


def kernel(x0: np.ndarray, x: np.ndarray, kernel: np.ndarray) -> np.ndarray:
    nc = _get_nc()
    in_maps = _in_maps(x0, x, kernel)
    res = run_bass_kernel_spmd(nc, in_maps, list(range(NCORES)))
    out = np.concatenate([np.asarray(r["out"]) for r in res.results], axis=0)
    return out.astype(np.float32)


# revision 12
# speedup vs baseline: 1.7414x; 1.0634x over previous
"""Trainium2 Bass kernel for CIN: out[b,m,d] = sigmoid(einsum('bid,bjd,ijm', x0, x, K)).

v11: uniform-K bf16 replication, engine-balanced evacuation.

Microbenchmark-derived rules baked in:
  - PE matmuls stream at ~216 ns/MM (N=512 warm) only when every MM has
    the SAME contraction row-config and dtype; K-mixing serializes to
    ~540 ns, fp32 moving operands pump at half rate AND lower to 2 HW
    passes. So: every matmul is bf16 with K=120.
  - PSUM-sourced elementwise ops run at 1x: DVE tensor_tensor 690 ns,
    ACT copy 720 ns per [*,512] chunk. The per-group broadcast evac
    (14 chunks) is split: 8 chunks DVE-direct, 5 chunks ACT-copy +
    GPSIMD-multiply, 1 chunk ACT-copy + DVE 2x multiply.

Per group (8 b's, free=512): 14 bf16 K=120 replication MMs (psa =
pp_c.T @ x0t broadcasts x0 rows across partitions), per-chunk multiply
ac = psa * xr on DVE/GP, 14 bf16 K=120 contraction MMs accumulate, ACT
sigmoid evacuation, DMA out. PE: 28 MMs/group = the pacer (~6 us).
"""

import sys

for _p in ("/opt/trn_rl_repo", "/root/.axon_site/_ro/trn_rl_repo"):
    if _p not in sys.path:
        sys.path.insert(0, _p)

from contextlib import ExitStack

import numpy as np
import ml_dtypes

import concourse.bass as bass
from concourse import bacc
import concourse.tile as tile
from concourse import mybir
from concourse.bass_utils import run_bass_kernel_spmd

B, F0, F, D, M = 4096, 40, 40, 64, 128
NCORES = 8
NB = B // NCORES            # 512
GB = 8
FREE = GB * D               # 512
NG = NB // GB               # 64
NCH = 14
CONTRACT = 120

f32 = mybir.dt.float32
bf16 = mybir.dt.bfloat16
BF16NP = ml_dtypes.bfloat16

GP_ROUTE = {2, 5, 8, 11}        # ACT-copy + GPSIMD multiply
ACTDVE_ROUTE = set()            # (ACT-copy+DVE-2x head-of-line blocks the DVE FIFO)
LOOKAHEAD = 3


def _pack_consts(kernel_np: np.ndarray):
    kc = np.zeros((CONTRACT, NCH, M), np.float32)
    pp = np.zeros((CONTRACT, NCH, 128), np.float32)
    for c in range(NCH):
        for t in range(3):
            i = 3 * c + t
            if i < F0:
                kc[t * 40:(t + 1) * 40, c, :] = kernel_np[i]
                pp[i, c, t * 40:(t + 1) * 40] = 1.0
    return kc.astype(BF16NP), pp.astype(BF16NP)


def _build():
    nc = bacc.Bacc("TRN2", num_devices=NCORES)
    x0h = nc.declare_dram_parameter("x0h", [F0, NG, FREE], bf16, isOutput=False)
    xh = nc.declare_dram_parameter("xh", [F, NG, FREE], bf16, isOutput=False)
    kch = nc.declare_dram_parameter("kch", [CONTRACT, NCH, M], bf16, isOutput=False)
    pph = nc.declare_dram_parameter("pph", [CONTRACT, NCH, 128], bf16, isOutput=False)
    outp = nc.declare_dram_parameter("out", [NB, M, D], f32, isOutput=True)

    with ExitStack() as ctx:
        tc = ctx.enter_context(tile.TileContext(nc))
        singles = ctx.enter_context(tc.tile_pool(name="singles", bufs=1))
        xr_pool = ctx.enter_context(tc.tile_pool(name="xr", bufs=3))
        x0_pool = ctx.enter_context(tc.tile_pool(name="x0", bufs=3))
        sb_pool = ctx.enter_context(tc.tile_pool(name="sb", bufs=6))
        a_pool = ctx.enter_context(tc.tile_pool(name="a", bufs=6))
        osb_pool = ctx.enter_context(tc.tile_pool(name="osb", bufs=3))
        psa_pool = ctx.enter_context(tc.tile_pool(name="psa", bufs=5, space="PSUM"))
        pso_pool = ctx.enter_context(tc.tile_pool(name="pso", bufs=3, space="PSUM"))

        kcw = singles.tile([CONTRACT, NCH, M], bf16)
        nc.sync.dma_start(out=kcw, in_=kch[:, :, :])
        ppw = singles.tile([CONTRACT, NCH, 128], bf16)
        nc.sync.dma_start(out=ppw, in_=pph[:, :, :])

        for g in range(NG):
            bsl = slice(g * GB, (g + 1) * GB)
            xr = xr_pool.tile([CONTRACT, FREE], bf16, tag="xr")
            for r in range(3):
                nc.sync.dma_start(out=xr[r * 40:(r + 1) * 40, :], in_=xh[:, g, :])
            # x0t rows 0-39 real; rows 40-119 finite copies (zero-weighted)
            x0t = x0_pool.tile([CONTRACT, FREE], bf16, tag="x0t")
            for r in range(3):
                nc.sync.dma_start(out=x0t[r * 40:(r + 1) * 40, :], in_=x0h[:, g, :])

            pso = pso_pool.tile([M, FREE], f32, tag="pso")
            acs = [None] * NCH
            for c in range(NCH + LOOKAHEAD):
                if c < NCH:
                    psa = psa_pool.tile([128, FREE], f32, tag="psa")
                    nc.tensor.matmul(psa, ppw[:, c, :], x0t, start=True, stop=True)
                    ac = a_pool.tile([CONTRACT, FREE], bf16, tag="ac")
                    if c in GP_ROUTE or c in ACTDVE_ROUTE:
                        sba = sb_pool.tile([CONTRACT, FREE], bf16, tag="sba")
                        nc.scalar.copy(sba, psa[0:CONTRACT, :])
                        eng = nc.gpsimd if c in GP_ROUTE else nc.vector
                        eng.tensor_tensor(out=ac, in0=sba, in1=xr,
                                          op=mybir.AluOpType.mult)
                    else:
                        nc.vector.tensor_tensor(out=ac, in0=psa[0:CONTRACT, :],
                                                in1=xr, op=mybir.AluOpType.mult)
                    acs[c] = ac
                if c >= LOOKAHEAD:
                    cc = c - LOOKAHEAD
                    nc.tensor.matmul(pso, kcw[:, cc, :], acs[cc],
                                     start=(cc == 0), stop=(cc == NCH - 1))

            osb = osb_pool.tile([M, GB, D], f32, tag="osb")
            nc.scalar.activation(osb.rearrange("m b d -> m (b d)"), pso,
                                 mybir.ActivationFunctionType.Sigmoid)
            nc.sync.dma_start(out=outp[bsl].transpose([1, 0, 2]), in_=osb)

    nc.finalize()
    return nc


_NC_CACHE = {}


def _get_nc():
    if "nc" not in _NC_CACHE:
        _NC_CACHE["nc"] = _build()
    return _NC_CACHE["nc"]


def _in_maps(x0: np.ndarray, x: np.ndarray, kernel_np: np.ndarray):
    kc, pp = _pack_consts(np.asarray(kernel_np, dtype=np.float32))
    x0 = np.asarray(x0, dtype=np.float32).astype(BF16NP)
    x = np.asarray(x, dtype=np.float32).astype(BF16NP)
    maps = []
    for i in range(NCORES):
        sl = slice(i * NB, (i + 1) * NB)
        x0c = np.ascontiguousarray(
            x0[sl].transpose(1, 0, 2).reshape(F0, NG, FREE))
        xc = np.ascontiguousarray(
            x[sl].transpose(1, 0, 2).reshape(F, NG, FREE))
        maps.append({"x0h": x0c, "xh": xc, "kch": kc, "pph": pp})
    return maps


def kernel(x0: np.ndarray, x: np.ndarray, kernel: np.ndarray) -> np.ndarray:
    nc = _get_nc()
    in_maps = _in_maps(x0, x, kernel)
    res = run_bass_kernel_spmd(nc, in_maps, list(range(NCORES)))
    out = np.concatenate([np.asarray(r["out"]) for r in res.results], axis=0)
    return out.astype(np.float32)
